# revision 30
# baseline (speedup 1.0000x reference)
"""Trainium2 Bass kernel for nn_NoduleRecallLoss (segment-reduce recall loss).

Computation (matches the reference):
    fg   = x[:, 1]                    # foreground logits [B,S,S,S]
    yb   = (y > 0) as float           # binary GT
    tp[s]    = sum over voxels with comp_labels==s of fg*yb
    tp_fn[s] = sum over voxels with comp_labels==s of yb
    recall = sum_{s=1..num_components} tp[s]/tp_fn[s]
    loss   = -(recall + 1) / (num_components + 1)

Strategy: data-parallel over 8 NeuronCores (flat voxel sharding).

Sentinel labels: host remaps lab' = lab if y>0 else 255.  All y=0 voxels
then fall in radix cell (lo=15, hi>=8) whose segment ids (>=143) are never
read, so the kernel needs neither y nor fg*y on device - the raw fg value
plane suffices.

Radix decomposition lab' = 16*hi + lo.  Per-segment sums factor:
    tp[16h+l] = sum_v d(lo=l)[v] * d(hi>=h)[v] * fg[v]   (cumulative in h)
The hi-side masks are built on the *scalar (ACT) engine* as
sign(lab' - 16h + 0.5) in {-1,+1}; since sum d*(2*ge-1)*w = 2*cum - total,
the host recovers cum from the sign-encoded columns using the h=0 columns.
The lo-side one-hots (15 is_equal ops) and the 8 sign*fg products are the
only DVE work; the 'ones' planes come from gpsimd memsets.

Tensor-engine schedule: per-chunk [128,16]x[128,18] matmuls pay a ~40ns
ldweights+dispatch toll each, so we pack G=8 chunks per matmul: stationary
= 8 chunks' lo-mask blocks [128, 16*8] (full 128-column weight load ->
fast-weight-load path), moving = the same 8 chunks' 18 value planes
[128, 18*8].  PSUM accumulates [128, 144] over all 1728 groups; only the
"diagonal" entries psum[l*8+c, j*8+c] are meaningful (off-diagonal cells
hold cross-chunk garbage that is ignored).  Host sums the diagonal blocks,
undoes the sign encoding and cumulative-h structure, and applies the loss.

Moving plane layout j: [0]=fg, [1..8]=sign_h*fg, [9]=ones, [10..17]=sign_h.
Stationary plane layout l: [0]=ones, [1..15]=(lo==l).
"""

import sys

sys.path.insert(0, "/opt/trn_rl_repo")

from contextlib import ExitStack

import numpy as np
import ml_dtypes

import concourse.bacc as bacc
import concourse.tile as tile
from concourse import mybir
from concourse.bass_utils import run_bass_kernel_spmd

# Problem geometry (hardcoded per spec).
B = 2
S = 192
NVOX = B * S * S * S  # 14,155,776
NCORES = 8
P = 128
V8 = NVOX // NCORES  # 1,769,472 voxels per core
FT = V8 // P  # 13,824 free columns per core
FTILE = 1152  # columns per tile
NT = FT // FTILE  # 12 tiles
NH = 9  # hi digit 0..8
NL = 16  # lo digit 0..15
NCOL = NH * 2  # 18 value planes per chunk
G = 8  # chunks per grouped matmul
NGRP = FTILE // G  # 96 groups per tile
SENT = 255.0  # sentinel label for y=0 voxels

_BF16 = mybir.dt.bfloat16
_F32 = mybir.dt.float32
_A = mybir.AluOpType
_ACT = mybir.ActivationFunctionType


def _build_program(reps=1):
    nc = bacc.Bacc("TRN2", target_bir_lowering=False)
    # const APs for the activation bias scalars (0.5 - 16h)
    for h in range(1, NH):
        v = 0.5 - 16.0 * h
        tns = nc.alloc_sbuf_tensor(f"const-bias-{h}", [128, 1], _F32)
        nc.gpsimd.memset(tns.ap(), v)
        nc.const_aps.aps[(_F32, v)] = tns.ap()
    nc.all_engine_barrier()
    fgd = nc.dram_tensor("fgd", [P, NT, FTILE], _BF16, kind="ExternalInput")
    slab = nc.dram_tensor("slab", [P, NT, FTILE], _BF16, kind="ExternalInput")
    slo = nc.dram_tensor("slo", [P, NT, FTILE], _BF16, kind="ExternalInput")
    # host-precomputed lo one-hot planes l=11..15, laid out group-interleaved
    # so one DMA per tile lands them inside the weights tile (80B runs)
    eqd = nc.dram_tensor(
        "eqd", [P, NT, FTILE // G, NL - 11, G], _BF16, kind="ExternalInput"
    )
    out = nc.dram_tensor("out", [P, NCOL * G], _F32, kind="ExternalOutput")

    with ExitStack() as ctx:
        tc = ctx.enter_context(tile.TileContext(nc))
        ins = ctx.enter_context(tc.tile_pool(name="ins", bufs=6))
        work = ctx.enter_context(tc.tile_pool(name="work", bufs=2))
        psum = ctx.enter_context(tc.tile_pool(name="psum", bufs=1, space="PSUM"))
        outp = ctx.enter_context(tc.tile_pool(name="outp", bufs=1))

        acc = psum.tile([P, NCOL * G], _F32)
        for rep in range(reps):
          for t in range(NT):
            slab_t = ins.tile([P, FTILE], _BF16, tag="slab")
            slo_t = ins.tile([P, FTILE], _BF16, tag="slo")
            nc.sync.dma_start(out=slab_t[:], in_=slab[:, t, :])
            nc.sync.dma_start(out=slo_t[:], in_=slo[:, t, :])

            # lhs is group-contiguous (weights APs must merge to one free
            # dim); rhs is plane-major so every producer writes contiguous
            # runs and the moving AP carries two free dims [NCOL, G].
            lhs = work.tile([P, NGRP, NL, G], _BF16, tag="lhs")
            rhs = work.tile([P, NCOL, FTILE], _BF16, tag="rhs")
            fg_p = rhs[:, 0, :]

            # ones planes (gpsimd) + fg value plane (contiguous DMA)
            nc.gpsimd.memset(lhs[:, :, 0, :], 1.0)
            nc.sync.dma_start(out=lhs[:, :, 11:NL, :], in_=eqd[:, t, :, :, :])
            nc.gpsimd.memset(rhs[:, NH, :], 1.0)
            nc.sync.dma_start(out=fg_p, in_=fgd[:, t, :])
            # hi-side sign masks on the scalar engine:
            #   sign(lab' - 16h + 0.5) = 2*(lab' >= 16h) - 1
            for h in range(1, NH):
                nc.scalar.activation(
                    out=rhs[:, NH + h, :], in_=slab_t[:], func=_ACT.Sign,
                    bias=0.5 - 16.0 * h, scale=1.0,
                )
            # lo one-hot masks l=1..10 (DVE tensor_scalar, 2x); l=11..15
            # arrive prebuilt from the host via DMA (idle-engine offload)
            for l in range(1, 11):
                nc.vector.tensor_scalar(
                    out=lhs[:, :, l, :], in0=slo_t[:], scalar1=float(l),
                    scalar2=None, op0=_A.is_equal,
                )
            # sign_h * fg products (DVE tensor_tensor)
            for h in range(1, NH):
                nc.vector.tensor_tensor(
                    out=rhs[:, h, :], in0=rhs[:, NH + h, :], in1=fg_p,
                    op=_A.mult,
                )
            # grouped matmuls accumulating into one PSUM region
            for g in range(NGRP):
                first = t == 0 and g == 0
                last = t == NT - 1 and g == NGRP - 1
                nc.tensor.matmul(
                    acc[:],
                    lhsT=lhs[:, g, :, :],
                    rhs=rhs[:, :, g * G:(g + 1) * G],
                    start=first, stop=last,
                )
        res = outp.tile([P, NCOL * G], _F32)
        nc.vector.tensor_copy(out=res[:], in_=acc[:])
        nc.sync.dma_start(out=out[:, :], in_=res[:])
    if not nc.is_finalized():
        nc.finalize()
    return nc


_PROGRAM = None


def _get_program():
    global _PROGRAM
    if _PROGRAM is None:
        _PROGRAM = _build_program()
    return _PROGRAM


def make_in_maps(x, y, comp_labels):
    """Host-side sharding + dtype staging (casts/packing only)."""
    bf16 = ml_dtypes.bfloat16
    fg = np.ascontiguousarray(x[:, 1]).reshape(-1).astype(bf16)
    yb = y.reshape(-1) > 0
    slab = np.where(yb, comp_labels.reshape(-1), 255).astype(np.int32)
    slo = (slab & 15).astype(bf16)
    slab = slab.astype(bf16)
    in_maps = []
    for c in range(NCORES):
        sl = slice(c * V8, (c + 1) * V8)
        slo_c = slo[sl].reshape(P, NT, FTILE)
        eq_c = (
            slo_c.reshape(P, NT, FTILE // G, 1, G)
            == np.arange(11, NL, dtype=bf16).reshape(1, 1, 1, NL - 11, 1)
        ).astype(bf16)
        in_maps.append(
            {
                "fgd": fg[sl].reshape(P, NT, FTILE),
                "slab": np.ascontiguousarray(slab[sl].reshape(P, NT, FTILE)),
                "slo": np.ascontiguousarray(slo_c),
                "eqd": np.ascontiguousarray(eq_c),
            }
        )
    return in_maps


def combine_partials(outs, num_components):
    """Sum per-core [128, 144] grouped partials -> loss scalar.

    psum[l*8+c', j*8+c]: diagonal blocks c'==c hold the real sums; summing
    them over c gives O[l, j] with plane semantics
      j=0: tp0[l] = sum d(lo=l)*fg          (over all voxels, cum_tp at h=0)
      j=1..8:  2*cum_tp[l,h] - tp0[l]       (sign encoding)
      j=9: cnt[l] = sum d(lo=l)             (cum_fn at h=0)
      j=10..17: 2*cum_fn[l,h] - cnt[l]
    Sentinel voxels (y=0 -> lab'=255, lo=15) sit in every cum_h, so they
    cancel in the h-differences and only pollute segment 143 (unused).
    """
    M = np.zeros((P, NCOL * G), np.float64)
    for o in outs:
        M += o.astype(np.float64)
    M = M.reshape(NL, G, NCOL, G)  # [l, c', j, c]
    O = np.einsum("lcjc->lj", M)   # sum diagonal blocks c'==c
    tp0 = O[:, 0]
    cnt = O[:, NH]
    cum_tp = np.zeros((NL, NH + 1))
    cum_fn = np.zeros((NL, NH + 1))
    cum_tp[:, 0] = tp0
    cum_fn[:, 0] = cnt
    for h in range(1, NH):
        cum_tp[:, h] = (O[:, h] + tp0) / 2.0
        cum_fn[:, h] = (O[:, NH + h] + cnt) / 2.0
    # cum at h=NH is zero (appended) -> difference to per-h cells
    Tc = cum_tp[:, :-1] - cum_tp[:, 1:]  # [NL, NH]
    Fc = cum_fn[:, :-1] - cum_fn[:, 1:]
    # undo missing row 0 (row 0 is the sum over all lo)
    T = np.zeros((NL, NH))
    F = np.zeros((NL, NH))
    T[1:, :] = Tc[1:, :]
    T[0, :] = Tc[0, :] - Tc[1:, :].sum(axis=0)
    F[1:, :] = Fc[1:, :]
    F[0, :] = Fc[0, :] - Fc[1:, :].sum(axis=0)
    tp = np.zeros(NL * NH, np.float64)
    tpfn = np.zeros(NL * NH, np.float64)
    for h in range(NH):
        for l in range(NL):
            tp[16 * h + l] = T[l, h]
            tpfn[16 * h + l] = F[l, h]
    n = int(num_components)
    with np.errstate(divide="ignore", invalid="ignore"):
        recall = np.sum(tp[1 : n + 1] / tpfn[1 : n + 1])
    loss = -(recall + 1.0) / (n + 1.0)
    return np.float32(loss)


def kernel(x, y, comp_labels, num_components):
    nc = _get_program()
    in_maps = make_in_maps(np.asarray(x), np.asarray(y), np.asarray(comp_labels))
    res = run_bass_kernel_spmd(nc, in_maps, list(range(NCORES)))
    outs = [res.results[c]["out"] for c in range(NCORES)]
    return combine_partials(outs, np.asarray(num_components))


# revision 31
# speedup vs baseline: 1.1035x; 1.1035x over previous
"""Trainium2 Bass kernel for nn_NoduleRecallLoss (segment-reduce recall loss).

Computation (matches the reference):
    fg   = x[:, 1]                    # foreground logits [B,S,S,S]
    yb   = (y > 0) as float           # binary GT
    tp[s]    = sum over voxels with comp_labels==s of fg*yb
    tp_fn[s] = sum over voxels with comp_labels==s of yb
    recall = sum_{s=1..num_components} tp[s]/tp_fn[s]
    loss   = -(recall + 1) / (num_components + 1)

Strategy: data-parallel over 8 NeuronCores (flat voxel sharding).

Sentinel labels: host remaps lab' = lab if y>0 else 255.  All y=0 voxels
then fall in radix cell (lo=15, hi>=8) whose segment ids (>=143) are never
read, so the kernel needs neither y nor fg*y on device - the raw fg value
plane suffices.

Radix decomposition lab' = 16*hi + lo.  Per-segment sums factor:
    tp[16h+l] = sum_v d(lo=l)[v] * d(hi>=h)[v] * fg[v]   (cumulative in h)
The hi-side masks are built on the *scalar (ACT) engine* as
sign(lab' - 16h + 0.5) in {-1,+1}; since sum d*(2*ge-1)*w = 2*cum - total,
the host recovers cum from the sign-encoded columns using the h=0 columns.
The lo-side one-hots (15 is_equal ops) and the 8 sign*fg products are the
only DVE work; the 'ones' planes come from gpsimd memsets.

Tensor-engine schedule: per-chunk [128,16]x[128,18] matmuls pay a ~40ns
ldweights+dispatch toll each, so we pack G=8 chunks per matmul: stationary
= 8 chunks' lo-mask blocks [128, 16*8] (full 128-column weight load ->
fast-weight-load path), moving = the same 8 chunks' 18 value planes
[128, 18*8].  PSUM accumulates [128, 144] over all 1728 groups; only the
"diagonal" entries psum[l*8+c, j*8+c] are meaningful (off-diagonal cells
hold cross-chunk garbage that is ignored).  Host sums the diagonal blocks,
undoes the sign encoding and cumulative-h structure, and applies the loss.

Moving plane layout j: [0]=fg, [1..8]=sign_h*fg, [9]=ones, [10..17]=sign_h.
Stationary plane layout l: [0]=ones, [1..15]=(lo==l).
"""

import sys

sys.path.insert(0, "/opt/trn_rl_repo")

from contextlib import ExitStack

import numpy as np
import ml_dtypes

import concourse.bacc as bacc
import concourse.tile as tile
from concourse import mybir
from concourse.bass_utils import run_bass_kernel_spmd

# Problem geometry (hardcoded per spec).
B = 2
S = 192
NVOX = B * S * S * S  # 14,155,776
NCORES = 8
P = 128
V8 = NVOX // NCORES  # 1,769,472 voxels per core
FT = V8 // P  # 13,824 free columns per core
FTILE = 1152  # columns per tile
NT = FT // FTILE  # 12 tiles
NH = 9  # hi digit 0..8
NL = 16  # lo digit 0..15
NCOL = NH * 2  # 18 value planes per chunk
G = 8  # chunks per grouped matmul
NGRP = FTILE // G  # 96 groups per tile
SENT = 255.0  # sentinel label for y=0 voxels

_BF16 = mybir.dt.bfloat16
_F32 = mybir.dt.float32
_A = mybir.AluOpType
_ACT = mybir.ActivationFunctionType


def _build_program(reps=1):
    nc = bacc.Bacc("TRN2", target_bir_lowering=False)
    # const APs for the activation bias scalars (0.5 - 16h)
    for h in range(1, NH):
        v = 0.5 - 16.0 * h
        tns = nc.alloc_sbuf_tensor(f"const-bias-{h}", [128, 1], _F32)
        nc.gpsimd.memset(tns.ap(), v)
        nc.const_aps.aps[(_F32, v)] = tns.ap()
    nc.all_engine_barrier()
    fgd = nc.dram_tensor("fgd", [P, NT, FTILE], _BF16, kind="ExternalInput")
    slab = nc.dram_tensor("slab", [P, NT, FTILE], _BF16, kind="ExternalInput")
    slo = nc.dram_tensor("slo", [P, NT, FTILE], _BF16, kind="ExternalInput")
    out = nc.dram_tensor("out", [P, NCOL * G], _F32, kind="ExternalOutput")

    with ExitStack() as ctx:
        tc = ctx.enter_context(tile.TileContext(nc))
        ins = ctx.enter_context(tc.tile_pool(name="ins", bufs=6))
        work = ctx.enter_context(tc.tile_pool(name="work", bufs=2))
        psum = ctx.enter_context(tc.tile_pool(name="psum", bufs=1, space="PSUM"))
        outp = ctx.enter_context(tc.tile_pool(name="outp", bufs=1))

        acc = psum.tile([P, NCOL * G], _F32)
        pending = None
        for rep in range(reps):
          for t in range(NT):
            slab_t = ins.tile([P, FTILE], _BF16, tag="slab")
            slo_t = ins.tile([P, FTILE], _BF16, tag="slo")
            nc.sync.dma_start(out=slo_t[:], in_=slo[:, t, :])
            nc.sync.dma_start(out=slab_t[:], in_=slab[:, t, :])

            # lhs is group-contiguous (weights APs must merge to one free
            # dim); rhs is plane-major so every producer writes contiguous
            # runs and the moving AP carries two free dims [NCOL, G].
            lhs = work.tile([P, NGRP, NL, G], _BF16, tag="lhs")
            rhs = work.tile([P, NCOL, FTILE], _BF16, tag="rhs")
            fg_p = rhs[:, 0, :]

            # ones planes (gpsimd) + fg value plane (contiguous DMA)
            nc.gpsimd.memset(lhs[:, :, 0, :], 1.0)
            nc.gpsimd.memset(rhs[:, NH, :], 1.0)
            nc.sync.dma_start(out=fg_p, in_=fgd[:, t, :])
            # hi-side sign masks on the scalar engine:
            #   sign(lab' - 16h + 0.5) = 2*(lab' >= 16h) - 1
            for h in range(1, NH):
                nc.scalar.activation(
                    out=rhs[:, NH + h, :], in_=slab_t[:], func=_ACT.Sign,
                    bias=0.5 - 16.0 * h, scale=1.0,
                )
            # lo one-hot masks (DVE tensor_scalar, 2x)
            for l in range(1, NL):
                nc.vector.tensor_scalar(
                    out=lhs[:, :, l, :], in0=slo_t[:], scalar1=float(l),
                    scalar2=None, op0=_A.is_equal,
                )
            # tile t's TT products + matmuls are emitted one tile late
            # (see flush below) so DVE can run tile t+1's eq planes while
            # the ACT signs of tile t drain
            def _flush(t, lhs, rhs, fg_p):
                for h in range(1, NH):
                    nc.vector.tensor_tensor(
                        out=rhs[:, h, :], in0=rhs[:, NH + h, :], in1=fg_p,
                        op=_A.mult,
                    )
                for g in range(NGRP):
                    first = t == 0 and g == 0
                    last = t == NT - 1 and g == NGRP - 1
                    nc.tensor.matmul(
                        acc[:],
                        lhsT=lhs[:, g, :, :],
                        rhs=rhs[:, :, g * G:(g + 1) * G],
                        start=first, stop=last,
                    )
            if pending is not None:
                _flush(*pending)
            pending = (t, lhs, rhs, fg_p)
        if pending is not None:
            _flush(*pending)
        res = outp.tile([P, NCOL * G], _F32)
        nc.vector.tensor_copy(out=res[:], in_=acc[:])
        nc.sync.dma_start(out=out[:, :], in_=res[:])
    if not nc.is_finalized():
        nc.finalize()
    return nc


_PROGRAM = None


def _get_program():
    global _PROGRAM
    if _PROGRAM is None:
        _PROGRAM = _build_program()
    return _PROGRAM


def make_in_maps(x, y, comp_labels):
    """Host-side sharding + dtype staging (casts/packing only)."""
    bf16 = ml_dtypes.bfloat16
    fg = np.ascontiguousarray(x[:, 1]).reshape(-1).astype(bf16)
    yb = y.reshape(-1) > 0
    slab = np.where(yb, comp_labels.reshape(-1), 255).astype(np.int32)
    slo = (slab & 15).astype(bf16)
    slab = slab.astype(bf16)
    in_maps = []
    for c in range(NCORES):
        sl = slice(c * V8, (c + 1) * V8)
        in_maps.append(
            {
                "fgd": fg[sl].reshape(P, NT, FTILE),
                "slab": np.ascontiguousarray(slab[sl].reshape(P, NT, FTILE)),
                "slo": np.ascontiguousarray(slo[sl].reshape(P, NT, FTILE)),
            }
        )
    return in_maps


def combine_partials(outs, num_components):
    """Sum per-core [128, 144] grouped partials -> loss scalar.

    psum[l*8+c', j*8+c]: diagonal blocks c'==c hold the real sums; summing
    them over c gives O[l, j] with plane semantics
      j=0: tp0[l] = sum d(lo=l)*fg          (over all voxels, cum_tp at h=0)
      j=1..8:  2*cum_tp[l,h] - tp0[l]       (sign encoding)
      j=9: cnt[l] = sum d(lo=l)             (cum_fn at h=0)
      j=10..17: 2*cum_fn[l,h] - cnt[l]
    Sentinel voxels (y=0 -> lab'=255, lo=15) sit in every cum_h, so they
    cancel in the h-differences and only pollute segment 143 (unused).
    """
    M = np.zeros((P, NCOL * G), np.float64)
    for o in outs:
        M += o.astype(np.float64)
    M = M.reshape(NL, G, NCOL, G)  # [l, c', j, c]
    O = np.einsum("lcjc->lj", M)   # sum diagonal blocks c'==c
    tp0 = O[:, 0]
    cnt = O[:, NH]
    cum_tp = np.zeros((NL, NH + 1))
    cum_fn = np.zeros((NL, NH + 1))
    cum_tp[:, 0] = tp0
    cum_fn[:, 0] = cnt
    for h in range(1, NH):
        cum_tp[:, h] = (O[:, h] + tp0) / 2.0
        cum_fn[:, h] = (O[:, NH + h] + cnt) / 2.0
    # cum at h=NH is zero (appended) -> difference to per-h cells
    Tc = cum_tp[:, :-1] - cum_tp[:, 1:]  # [NL, NH]
    Fc = cum_fn[:, :-1] - cum_fn[:, 1:]
    # undo missing row 0 (row 0 is the sum over all lo)
    T = np.zeros((NL, NH))
    F = np.zeros((NL, NH))
    T[1:, :] = Tc[1:, :]
    T[0, :] = Tc[0, :] - Tc[1:, :].sum(axis=0)
    F[1:, :] = Fc[1:, :]
    F[0, :] = Fc[0, :] - Fc[1:, :].sum(axis=0)
    tp = np.zeros(NL * NH, np.float64)
    tpfn = np.zeros(NL * NH, np.float64)
    for h in range(NH):
        for l in range(NL):
            tp[16 * h + l] = T[l, h]
            tpfn[16 * h + l] = F[l, h]
    n = int(num_components)
    with np.errstate(divide="ignore", invalid="ignore"):
        recall = np.sum(tp[1 : n + 1] / tpfn[1 : n + 1])
    loss = -(recall + 1.0) / (n + 1.0)
    return np.float32(loss)


def kernel(x, y, comp_labels, num_components):
    nc = _get_program()
    in_maps = make_in_maps(np.asarray(x), np.asarray(y), np.asarray(comp_labels))
    res = run_bass_kernel_spmd(nc, in_maps, list(range(NCORES)))
    outs = [res.results[c]["out"] for c in range(NCORES)]
    return combine_partials(outs, np.asarray(num_components))


# revision 32
# speedup vs baseline: 1.3607x; 1.2330x over previous
"""Trainium2 Bass kernel for nn_NoduleRecallLoss (segment-reduce recall loss).

Computation (matches the reference):
    fg   = x[:, 1]                    # foreground logits [B,S,S,S]
    yb   = (y > 0) as float           # binary GT
    tp[s]    = sum over voxels with comp_labels==s of fg*yb
    tp_fn[s] = sum over voxels with comp_labels==s of yb
    recall = sum_{s=1..num_components} tp[s]/tp_fn[s]
    loss   = -(recall + 1) / (num_components + 1)

Strategy: data-parallel over 8 NeuronCores (flat voxel sharding).

Sentinel labels: host remaps lab' = lab if y>0 else 255.  All y=0 voxels
then fall in radix cell (lo=15, hi>=8) whose segment ids (>=143) are never
read, so the kernel needs neither y nor fg*y on device - the raw fg value
plane suffices.

Radix decomposition lab' = 16*hi + lo.  Per-segment sums factor:
    tp[16h+l] = sum_v d(lo=l)[v] * d(hi>=h)[v] * fg[v]   (cumulative in h)
The hi-side masks are built on the *scalar (ACT) engine* as
sign(lab' - 16h + 0.5) in {-1,+1}; since sum d*(2*ge-1)*w = 2*cum - total,
the host recovers cum from the sign-encoded columns using the h=0 columns.
The lo-side one-hots (15 is_equal ops) and the 8 sign*fg products are the
only DVE work; the 'ones' planes come from gpsimd memsets.

Tensor-engine schedule: per-chunk [128,16]x[128,18] matmuls pay a ~40ns
ldweights+dispatch toll each, so we pack G=8 chunks per matmul: stationary
= 8 chunks' lo-mask blocks [128, 16*8] (full 128-column weight load ->
fast-weight-load path), moving = the same 8 chunks' 18 value planes
[128, 18*8].  PSUM accumulates [128, 144] over all 1728 groups; only the
"diagonal" entries psum[l*8+c, j*8+c] are meaningful (off-diagonal cells
hold cross-chunk garbage that is ignored).  Host sums the diagonal blocks,
undoes the sign encoding and cumulative-h structure, and applies the loss.

Moving plane layout j: [0]=fg, [1..8]=sign_h*fg, [9]=ones, [10..17]=sign_h.
Stationary plane layout l: [0]=ones, [1..15]=(lo==l).
"""

import sys

sys.path.insert(0, "/opt/trn_rl_repo")

from contextlib import ExitStack

import numpy as np
import ml_dtypes

import concourse.bacc as bacc
import concourse.tile as tile
from concourse import mybir
from concourse.bass_utils import run_bass_kernel_spmd

# Problem geometry (hardcoded per spec).
B = 2
S = 192
NVOX = B * S * S * S  # 14,155,776
NCORES = 8
P = 128
V8 = NVOX // NCORES  # 1,769,472 voxels per core
FT = V8 // P  # 13,824 free columns per core
FTILE = 1152  # columns per tile
NT = FT // FTILE  # 12 tiles
NH = 9  # hi digit 0..8
NL = 16  # lo digit 0..15
NCOL = NH * 2  # 18 value planes per chunk
G = 8  # chunks per grouped matmul
NGRP = FTILE // G  # 96 groups per tile
SENT = 255.0  # sentinel label for y=0 voxels

_BF16 = mybir.dt.bfloat16
_F32 = mybir.dt.float32
_A = mybir.AluOpType
_ACT = mybir.ActivationFunctionType


def _build_program(reps=1):
    nc = bacc.Bacc("TRN2", target_bir_lowering=False)
    # const APs for the activation bias scalars (0.5 - 16h)
    for h in range(1, NH):
        v = 0.5 - 16.0 * h
        tns = nc.alloc_sbuf_tensor(f"const-bias-{h}", [128, 1], _F32)
        nc.gpsimd.memset(tns.ap(), v)
        nc.const_aps.aps[(_F32, v)] = tns.ap()
    nc.all_engine_barrier()
    fgd = nc.dram_tensor("fgd", [P, NT, FTILE], _BF16, kind="ExternalInput")
    slab = nc.dram_tensor("slab", [P, NT, FTILE], _BF16, kind="ExternalInput")
    slo = nc.dram_tensor("slo", [P, NT, FTILE], _BF16, kind="ExternalInput")
    out = nc.dram_tensor("out", [P, NCOL * G], _F32, kind="ExternalOutput")

    with ExitStack() as ctx:
        tc = ctx.enter_context(tile.TileContext(nc))
        ins = ctx.enter_context(tc.tile_pool(name="ins", bufs=6))
        work = ctx.enter_context(tc.tile_pool(name="work", bufs=2))
        psum = ctx.enter_context(tc.tile_pool(name="psum", bufs=1, space="PSUM"))
        outp = ctx.enter_context(tc.tile_pool(name="outp", bufs=1))

        acc = psum.tile([P, NCOL * G], _F32)
        for rep in range(reps):
          for t in range(NT):
            slab_t = ins.tile([P, FTILE], _BF16, tag="slab")
            slo_t = ins.tile([P, FTILE], _BF16, tag="slo")
            nc.sync.dma_start(out=slo_t[:], in_=slo[:, t, :])
            nc.sync.dma_start(out=slab_t[:], in_=slab[:, t, :])

            # lhs is group-contiguous (weights APs must merge to one free
            # dim); rhs is plane-major so every producer writes contiguous
            # runs and the moving AP carries two free dims [NCOL, G].
            lhs = work.tile([P, NGRP, NL, G], _BF16, tag="lhs")
            rhs = work.tile([P, NCOL, FTILE], _BF16, tag="rhs")
            fg_p = rhs[:, 0, :]

            # ones planes (gpsimd) + fg value plane (contiguous DMA)
            nc.gpsimd.memset(lhs[:, :, 0, :], 1.0)
            nc.gpsimd.memset(rhs[:, NH, :], 1.0)
            nc.sync.dma_start(out=fg_p, in_=fgd[:, t, :])
            # hi-side sign masks on the scalar engine:
            #   sign(lab' - 16h + 0.5) = 2*(lab' >= 16h) - 1
            for h in range(1, NH):
                nc.scalar.activation(
                    out=rhs[:, NH + h, :], in_=slab_t[:], func=_ACT.Sign,
                    bias=0.5 - 16.0 * h, scale=1.0,
                )
            # lo one-hot masks (DVE tensor_scalar, 2x)
            for l in range(1, NL):
                nc.vector.tensor_scalar(
                    out=lhs[:, :, l, :], in0=slo_t[:], scalar1=float(l),
                    scalar2=None, op0=_A.is_equal,
                )
            # sign_h * fg products (DVE tensor_tensor)
            for h in range(1, NH):
                nc.vector.tensor_tensor(
                    out=rhs[:, h, :], in0=rhs[:, NH + h, :], in1=fg_p,
                    op=_A.mult,
                )
            # grouped matmuls accumulating into one PSUM region
            for g in range(NGRP):
                first = t == 0 and g == 0
                last = t == NT - 1 and g == NGRP - 1
                nc.tensor.matmul(
                    acc[:],
                    lhsT=lhs[:, g, :, :],
                    rhs=rhs[:, :, g * G:(g + 1) * G],
                    start=first, stop=last,
                )
        res = outp.tile([P, NCOL * G], _F32)
        nc.vector.tensor_copy(out=res[:], in_=acc[:])
        nc.sync.dma_start(out=out[:, :], in_=res[:])
    if not nc.is_finalized():
        nc.finalize()
    return nc


_PROGRAM = None


def _get_program():
    global _PROGRAM
    if _PROGRAM is None:
        _PROGRAM = _build_program()
    return _PROGRAM


def make_in_maps(x, y, comp_labels):
    """Host-side sharding + dtype staging (casts/packing only)."""
    bf16 = ml_dtypes.bfloat16
    fg = np.ascontiguousarray(x[:, 1]).reshape(-1).astype(bf16)
    yb = y.reshape(-1) > 0
    slab = np.where(yb, comp_labels.reshape(-1), 255).astype(np.int32)
    slo = (slab & 15).astype(bf16)
    slab = slab.astype(bf16)
    in_maps = []
    for c in range(NCORES):
        sl = slice(c * V8, (c + 1) * V8)
        in_maps.append(
            {
                "fgd": fg[sl].reshape(P, NT, FTILE),
                "slab": np.ascontiguousarray(slab[sl].reshape(P, NT, FTILE)),
                "slo": np.ascontiguousarray(slo[sl].reshape(P, NT, FTILE)),
            }
        )
    return in_maps


def combine_partials(outs, num_components):
    """Sum per-core [128, 144] grouped partials -> loss scalar.

    psum[l*8+c', j*8+c]: diagonal blocks c'==c hold the real sums; summing
    them over c gives O[l, j] with plane semantics
      j=0: tp0[l] = sum d(lo=l)*fg          (over all voxels, cum_tp at h=0)
      j=1..8:  2*cum_tp[l,h] - tp0[l]       (sign encoding)
      j=9: cnt[l] = sum d(lo=l)             (cum_fn at h=0)
      j=10..17: 2*cum_fn[l,h] - cnt[l]
    Sentinel voxels (y=0 -> lab'=255, lo=15) sit in every cum_h, so they
    cancel in the h-differences and only pollute segment 143 (unused).
    """
    M = np.zeros((P, NCOL * G), np.float64)
    for o in outs:
        M += o.astype(np.float64)
    M = M.reshape(NL, G, NCOL, G)  # [l, c', j, c]
    O = np.einsum("lcjc->lj", M)   # sum diagonal blocks c'==c
    tp0 = O[:, 0]
    cnt = O[:, NH]
    cum_tp = np.zeros((NL, NH + 1))
    cum_fn = np.zeros((NL, NH + 1))
    cum_tp[:, 0] = tp0
    cum_fn[:, 0] = cnt
    for h in range(1, NH):
        cum_tp[:, h] = (O[:, h] + tp0) / 2.0
        cum_fn[:, h] = (O[:, NH + h] + cnt) / 2.0
    # cum at h=NH is zero (appended) -> difference to per-h cells
    Tc = cum_tp[:, :-1] - cum_tp[:, 1:]  # [NL, NH]
    Fc = cum_fn[:, :-1] - cum_fn[:, 1:]
    # undo missing row 0 (row 0 is the sum over all lo)
    T = np.zeros((NL, NH))
    F = np.zeros((NL, NH))
    T[1:, :] = Tc[1:, :]
    T[0, :] = Tc[0, :] - Tc[1:, :].sum(axis=0)
    F[1:, :] = Fc[1:, :]
    F[0, :] = Fc[0, :] - Fc[1:, :].sum(axis=0)
    tp = np.zeros(NL * NH, np.float64)
    tpfn = np.zeros(NL * NH, np.float64)
    for h in range(NH):
        for l in range(NL):
            tp[16 * h + l] = T[l, h]
            tpfn[16 * h + l] = F[l, h]
    n = int(num_components)
    with np.errstate(divide="ignore", invalid="ignore"):
        recall = np.sum(tp[1 : n + 1] / tpfn[1 : n + 1])
    loss = -(recall + 1.0) / (n + 1.0)
    return np.float32(loss)


def kernel(x, y, comp_labels, num_components):
    nc = _get_program()
    in_maps = make_in_maps(np.asarray(x), np.asarray(y), np.asarray(comp_labels))
    res = run_bass_kernel_spmd(nc, in_maps, list(range(NCORES)))
    outs = [res.results[c]["out"] for c in range(NCORES)]
    return combine_partials(outs, np.asarray(num_components))
